# revision 72
# baseline (speedup 1.0000x reference)
"""Trainium2 Bass kernel for multi-head attention (B=4, L=2048, D=1024, H=16).

Sharding: 8 cores = 4 batches x 2 head-groups (8 heads each).
Per core: QKV projection (its head slice), RoPE, per-head attention,
output projection against its w_out column slice; host sums the two
per-batch partials.

Design (cost-model driven; ACT/exp at ~255us busy is the bottleneck,
PE ~225us after the fp8 scores, so everything schedules around ACT):
- 32 iterations, one (head, q-chunk) each; scores for 16 k-tiles in
  groups of (3,3,3,3,2,2) -> PSUM [128,1536] x2 bufs -> exp on ACT into
  an SBUF tile [128,16,512] (2 buffers).
- Scores matmuls run in fp8(e4m3) DoubleRow mode at 0.5 cycles/row:
  operands keep the plain layout (hd on partitions); the DoubleRow
  second K-tile plane (j=1) is all zeros, so the result is the exact
  hd=64 dot product at half the per-row cost.  RoPE writes the fp8
  operand tiles directly (quantization error ~1.4e-2 rel, under the
  2e-2 budget).
- PV is q-major: out [128 q, 65] = expS[128 m, 128 q].T @ Vones[128 m, 65];
  one q-tile accumulator at a time in a single PSUM bank (PSUM
  accumulation groups must own a full 2KB bank granule), sequenced as
  4 passes with the softmax denominator in the 65th column.  PV for
  iteration i runs during iteration i+1.
- Normalized [q, 64] halves of a head pair pack into [128, 128] tiles
  and move to feature-major cT via DMA xbar transposes (no engine cost).
- QKV projection, RoPE pieces, and the output projection are pending
  units (~0.5-0.9us PE each) popped between score groups against
  just-in-time (iteration, group) deadlines.
- Prologue: head pair 0's q/k projection uses 8 concurrent full-bank
  PSUM accumulators with the K-chunk loop outermost so PE starts as
  soon as xT chunk 0 lands and the p-state ramp is preserved.

PSUM banks: 0-5 scores (2x [128,1536]), 6 pv, 7 projection pieces.
Matmuls bf16 except fp8 scores; fp32 PSUM; fp32 output.
"""
import sys

sys.path.insert(0, "/opt/trn_rl_repo")
import numpy as np
import concourse.bass as bass
import concourse.bacc as bacc
import concourse.mybir as mybir
from concourse.tile import TileContext
from concourse.bass_utils import run_bass_kernel_spmd

L = 2048          # sequence length
D = 1024          # model dim
HD = 64           # head dim
KT = L // 128     # 16 k position tiles
QC = 4            # q chunks of 512
DT = mybir.dt.bfloat16
F32 = mybir.dt.float32
SCALE = HD ** -0.5
AF = mybir.ActivationFunctionType
GROUPS = [(0, 3), (3, 3), (6, 3), (9, 3), (12, 2), (14, 2)]


def build_nc():
    nc = bacc.Bacc("TRN2", target_bir_lowering=False, debug=False, num_devices=8)
    xT = nc.dram_tensor("xT", [D, L], DT, kind="ExternalInput")
    wqkT = nc.dram_tensor("wqkT", [8, 128, 8, 128], DT, kind="ExternalInput")
    wvT = nc.dram_tensor("wvT", [D, 512], DT, kind="ExternalInput")
    bqk = nc.dram_tensor("bqk", [128, 8], F32, kind="ExternalInput")
    bv = nc.dram_tensor("bv", [1, 512], DT, kind="ExternalInput")
    woT = nc.dram_tensor("woT", [512, D], DT, kind="ExternalInput")
    bout = nc.dram_tensor("bout", [1, D], DT, kind="ExternalInput")
    z8 = nc.dram_tensor("z8", [128, L], mybir.dt.float8e4, kind="ExternalInput")
    cosT = nc.dram_tensor("cosT", [128, L], DT, kind="ExternalInput")
    sinT = nc.dram_tensor("sinT", [128, L], DT, kind="ExternalInput")
    out = nc.dram_tensor("out", [L, D], DT, kind="ExternalOutput")

    with TileContext(nc) as tc:
        with (
            tc.tile_pool(name="const", bufs=1) as cp,
            tc.tile_pool(name="wstream", bufs=2) as wsp,
            tc.tile_pool(name="rope", bufs=4) as rp,
            tc.tile_pool(name="exps", bufs=2) as ep,
            tc.tile_pool(name="ctile", bufs=8) as ctp,
            tc.tile_pool(name="small", bufs=8) as sp,
            tc.tile_pool(name="psum", bufs=1, space="PSUM") as pp,
        ):
            dma = nc.default_dma_engine     # SP queue (HWDGE)
            dma2 = nc.gpsimd                # Pool queue (SWDGE)

            # ---- resident inputs, strict priority order on one queue:
            # xT0, xT1, first wqk pair, cos/sin (rope), then the rest ----
            xT_sb = cp.tile([128, 8, L], DT)
            FP8 = mybir.dt.float8e4
            # fp8 scores operands; j=1 plane zeros (DoubleRow zero-pad)
            qk8 = [cp.tile([128, 2, L], FP8, name=f"qk8_{c}") for c in range(8)]
            wqk_tiles = {}
            for fc in (0, 4):
                wqk_tiles[fc] = wsp.tile([128, 8, 128], DT, tag="wqk",
                                         name=f"wqk{fc}")
            dma.dma_start(out=xT_sb[:, 0, :], in_=xT[0:128, :])
            dma.dma_start(out=xT_sb[:, 1, :], in_=xT[128:256, :])
            dma.dma_start(out=wqk_tiles[0][:], in_=wqkT[0])
            dma.dma_start(out=wqk_tiles[4][:], in_=wqkT[4])
            cos_sb = cp.tile([128, L], DT)
            dma.dma_start(out=cos_sb[:], in_=cosT[:])
            sin_sb = cp.tile([128, L], DT)
            dma.dma_start(out=sin_sb[:], in_=sinT[:])
            bqk_sb = cp.tile([128, 8], F32)
            dma.dma_start(out=bqk_sb[:], in_=bqk[:])
            for c in range(2, 8):
                dma.dma_start(out=xT_sb[:, c, :], in_=xT[c * 128:(c + 1) * 128, :])
            wvT_sb = cp.tile([128, 8, 512], DT)
            for c in range(8):
                dma.dma_start(out=wvT_sb[:, c, :], in_=wvT[c * 128:(c + 1) * 128, :])
            bv_sb = cp.tile([1, 512], DT)
            dma.dma_start(out=bv_sb[:], in_=bv[:])
            woT_sb = cp.tile([128, 4, D], DT)
            for c in range(4):
                dma.dma_start(out=woT_sb[:, c, :], in_=woT[c * 128:(c + 1) * 128, :])
            bout_sb = cp.tile([1, D], DT)
            dma.dma_start(out=bout_sb[:], in_=bout[:])
            bv_bc = cp.tile([128, 512], DT)
            bout_bc = cp.tile([128, D], DT)

            qkc = {}    # transient biased chunk tiles (bias -> rot/rope -> dead)

            def qk_chunk(c):
                if c not in qkc:
                    qkc[c] = rp.tile([128, L], DT, tag="qkc", bufs=3,
                                     name=f"qkc{c}")
                return qkc[c]
            V_sb = cp.tile([128, KT, 8 * (HD + 1)], DT)  # V + ones col per head
            cT = [cp.tile([128, 4, 512], DT, name=f"cT{qc}") for qc in range(QC)]
            pvT = pp.tile([128, 512], F32, tag="pv", name="pv")
            pjT = pp.tile([128, 512], F32, tag="pj", name="pj")
            wide = [None, 0, False]   # [tile, slot, enabled]

            def proj_slot():
                # one accumulation group per full PSUM bank granule (512 fp32)
                if not wide[2]:
                    return pjT[:]
                if wide[0] is None or wide[1] == 3:
                    wide[0] = pp.tile([128, 1536], F32, tag="s", bufs=2,
                                      name="wides")
                    wide[1] = 0
                s = wide[0][:, wide[1] * 512:wide[1] * 512 + 512]
                wide[1] += 1
                return s

            # ---- pending unit emitters ----
            def qk_proj_unit(fc, nt):
                def emit():
                    if fc not in wqk_tiles:
                        wqk_tiles[fc] = wsp.tile([128, 8, 128], DT, tag="wqk",
                                                 name=f"wqk{fc}")
                        dma.dma_start(out=wqk_tiles[fc][:], in_=wqkT[fc])
                    wt = wqk_tiles[fc]
                    ps = proj_slot()
                    lo = nt * 512
                    for kc in range(8):
                        nc.tensor.matmul(ps[:], lhsT=wt[:, kc, :],
                                         rhs=xT_sb[:, kc, lo:lo + 512],
                                         start=(kc == 0), stop=(kc == 7))
                    nc.vector.tensor_scalar_add(qk_chunk(fc)[:, lo:lo + 512],
                                                ps[:], bqk_sb[:, fc:fc + 1])
                return emit

            rot_chunks = {}

            def rot_unit(c):
                # partition-swapped copy of chunk c (rot-half trick), done as
                # 4 whole-chunk DMAs right after the chunk's bias-adds so the
                # copies never wait at the head of the SP queue.
                def emit():
                    rot = rp.tile([128, L], DT, tag="rotc", bufs=4,
                                  name=f"rotc{c}")
                    rot_chunks[c] = rot
                    for h2 in range(2):
                        p = 64 * h2
                        dma.dma_start(out=rot[p:p + 32, :],
                                      in_=qk_chunk(c)[p + 32:p + 64, :])
                        dma.dma_start(out=rot[p + 32:p + 64, :],
                                      in_=qk_chunk(c)[p:p + 32, :])
                return emit

            def rope_piece(c, lo, w):
                # RoPE on qkT[:, c, lo:lo+w]; rot-half sign baked into sinT;
                # DVE-only (rot copy staged by rot_unit(c))
                def emit():
                    rot = rot_chunks[c]
                    tmp = rp.tile([128, 512], DT, tag="ropetmp", name=f"rt{c}_{lo}")
                    nc.vector.tensor_mul(tmp[:, 0:w], qk_chunk(c)[:, lo:lo + w],
                                         cos_sb[:, lo:lo + w])
                    nc.vector.tensor_mul(rot[:, lo:lo + w], rot[:, lo:lo + w],
                                         sin_sb[:, lo:lo + w])
                    nc.vector.tensor_add(qk8[c][:, 0, lo:lo + w], tmp[:, 0:w],
                                         rot[:, lo:lo + w])
                return emit

            def v_proj_unit(lt):
                def emit():
                    ps = proj_slot()
                    for kc in range(8):
                        nc.tensor.matmul(ps[:],
                                         lhsT=xT_sb[:, kc, lt * 128:(lt + 1) * 128],
                                         rhs=wvT_sb[:, kc, :],
                                         start=(kc == 0), stop=(kc == 7))
                    v4 = V_sb[:, lt, :].rearrange("p (h c) -> p h c", c=HD + 1)
                    nc.vector.tensor_add(
                        v4[:, :, 0:HD],
                        ps[:].rearrange("p (h c) -> p h c", c=HD),
                        bv_bc[:].rearrange("p (h c) -> p h c", c=HD))
                    nc.vector.memset(v4[:, :, HD:HD + 1], 1.0)
                return emit

            def op_unit(qc, mq, half):
                # out-proj Y[q128, 512] for q tile mq of qc, dout half
                def emit():
                    osb = ctp.tile([128, 512], DT, tag="osb", name=f"osb{qc}{mq}{half}")
                    ps = proj_slot()
                    for cc in range(4):
                        nc.tensor.matmul(
                            ps[:],
                            lhsT=cT[qc][:, cc, mq * 128:(mq + 1) * 128],
                            rhs=woT_sb[:, cc, half * 512:(half + 1) * 512],
                            start=(cc == 0), stop=(cc == 3))
                    nc.vector.tensor_add(osb[:], ps[:],
                                         bout_bc[:, half * 512:(half + 1) * 512])
                    (dma if wide[2] else dma2).dma_start(
                        out=out[qc * 512 + mq * 128:qc * 512 + (mq + 1) * 128,
                                half * 512:(half + 1) * 512],
                        in_=osb[:])
                return emit

            # ---- pending queue: (deadline (iter, group), emit) ----
            KCOL = [(k0 * 128, n * 128) for k0, n in GROUPS]   # col ranges/group
            pending = []
            # k-chunk-4 rope pieces 1..5 (piece 0 done in prologue), JIT
            for gi in range(1, 6):
                lo, w = KCOL[gi]
                pending.append(((0, gi - 1), rope_piece(4, lo, w)))
            # q-chunk-0 rope pieces for qc=1..3
            for q in range(1, 4):
                pending.append(((2 * q - 1, 3), rope_piece(0, q * 512, 512)))
            # V projection: needed by pv(iter0) emitted during iter 1
            gi_of_kt = {}
            for _gi, (_k0, _n) in enumerate(GROUPS):
                for _kt in range(_k0, _k0 + _n):
                    gi_of_kt[_kt] = _gi
            for lt in range(KT):
                if lt < 8:
                    dl = (0, min(3, gi_of_kt[lt]))
                elif lt < 12:
                    dl = (1, 0)
                else:
                    dl = (1, 1)
                pending.append((dl, v_proj_unit(lt)))
            # later head pairs: qk projection + rope pieces
            for hp in range(1, 4):
                base = hp * 8
                for nt in range(4):
                    pending.append(((base - 6 + nt, 0), qk_proj_unit(hp, nt)))
                    pending.append(((base - 6 + nt, 3), qk_proj_unit(4 + hp, nt)))
                pending.append(((base - 2, 4), rot_unit(hp)))
                pending.append(((base - 2, 4), rot_unit(4 + hp)))
                for gi in range(6):
                    lo, w = KCOL[gi]
                    dl = (base - 1, 5) if gi == 0 else (base, gi - 1)
                    pending.append((dl, rope_piece(4 + hp, lo, w)))
                for q in range(4):
                    pending.append(((base - 1, q), rope_piece(hp, q * 512, 512)))
            pending.sort(key=lambda u: u[0])

            def pop(n, it):
                for _ in range(n):
                    if pending and pending[0][0] <= (it + 3, 5):
                        pending.pop(0)[1]()

            def drain(it, gi):
                while pending and pending[0][0] <= (it, gi):
                    pending.pop(0)[1]()

            # ---- prologue: head pair 0's q,k projection, 8 concurrent
            # bank accumulators, kc outermost; then rope pieces for the
            # first iteration's operands ----
            pro_s = [pp.tile([128, 1536], F32, tag="s", bufs=2, name=f"pro{i}")
                     for i in range(2)]
            pro_slots = [pro_s[0][:, 0:512], pro_s[0][:, 512:1024],
                         pro_s[0][:, 1024:1536], pro_s[1][:, 0:512],
                         pro_s[1][:, 512:1024], pro_s[1][:, 1024:1536],
                         pjT[:], pvT[:]]
            PRO_U = [(0, 0), (4, 0), (0, 1), (4, 1), (0, 2), (4, 2), (0, 3), (4, 3)]
            rot_chunks[0] = rp.tile([128, L], DT, tag="rotc", bufs=4, name="rotc0")
            rot_chunks[4] = rp.tile([128, L], DT, tag="rotc", bufs=4, name="rotc4")

            def pro_bias_rot(u):
                # u0/u1 gate the rope chain: run them on the still-idle ACT
                # engine so DVE starts rope immediately; the rest stay on DVE.
                fc, nt = PRO_U[u]
                eng = nc.scalar.add if u < 6 else nc.vector.tensor_scalar_add
                eng(qk_chunk(fc)[:, nt * 512:(nt + 1) * 512],
                    pro_slots[u], bqk_sb[:, fc:fc + 1])
                lo = nt * 512
                rot = rot_chunks[fc]
                for h2 in range(2):
                    p = 64 * h2
                    dma.dma_start(out=rot[p:p + 32, lo:lo + 512],
                                  in_=qk_chunk(fc)[p + 32:p + 64, lo:lo + 512])
                    dma.dma_start(out=rot[p + 32:p + 64, lo:lo + 512],
                                  in_=qk_chunk(fc)[p:p + 32, lo:lo + 512])

            for c8 in (0, 4, 1, 5, 2, 6, 3, 7):
                dma.dma_start(out=qk8[c8][:, 1, :], in_=z8[:])
            nc.gpsimd.partition_broadcast(bv_bc[:], bv_sb[:])
            nc.gpsimd.partition_broadcast(bout_bc[:], bout_sb[:])
            for kc in range(8):
                for u, (fc, nt) in enumerate(PRO_U):
                    nc.tensor.matmul(pro_slots[u],
                                     lhsT=wqk_tiles[fc][:, kc, :],
                                     rhs=xT_sb[:, kc, nt * 512:(nt + 1) * 512],
                                     start=(kc == 0), stop=(kc == 7),
                                     skip_group_check=True)
                    if kc == 7 and u < 2:
                        pro_bias_rot(u)
            rope_piece(0, 0, 512)()
            rope_piece(4, 0, 384)()
            for u in range(2, 8):
                pro_bias_rot(u)

            def emit_pv_qt(h2, hp, exp_t, qt, kts, bank):
                h = 2 * hp + h2
                for kt in kts:
                    nc.tensor.matmul(
                        bank[:, 0:65],
                        lhsT=exp_t[:, kt, qt * 128:(qt + 1) * 128],
                        rhs=V_sb[:, kt, h * 65:(h + 1) * 65],
                        start=(kt == 0), stop=(kt == KT - 1),
                        skip_group_check=True)

            nO_tiles = {}

            def normalize_qt(ph2, php, pqc, qt, bank):
                # one qt accumulator -> normalized nO half -> (h2==1) cT
                rrec = sp.tile([128, 1], F32, tag="rrec", name=f"rr{qt}")
                nc.vector.reciprocal(rrec[:], bank[:, 64:65])
                if ph2 == 0:
                    nO_tiles[qt] = sp.tile([128, 128], DT, tag="nO",
                                           name=f"nO{php}_{pqc}_{qt}")
                nO = nO_tiles[qt]
                nc.vector.tensor_scalar_mul(
                    nO[:, ph2 * 64:ph2 * 64 + 64], bank[:, 0:64], rrec[:])
                if ph2 == 1:
                    dma.dma_start_transpose(
                        cT[pqc][:, php, qt * 128:(qt + 1) * 128], nO[:])
                if ph2 == 1 and php == 3 and qt == 3:
                    for mq in range(4):
                        for half in range(2):
                            pending.append(((27 + 2 * pqc, mq + half), op_unit(pqc, mq, half)))

            # ---- main loop ----
            prev = None
            PV_PACE = (2, 2, 1, 1, 0, 0)
            for it in range(32):
                hp, r = divmod(it, 8)
                qc, h2 = divmod(r, 2)
                drain(it, 0)
                exp_t = ep.tile([128, KT, 512], DT, tag="exp", bufs=2)
                p = 64 * h2
                for gi, (k0, n) in enumerate(GROUPS):
                    sps = pp.tile([128, 1536], F32, tag="s", bufs=2, name="sps")
                    for j in range(n):
                        kt = k0 + j
                        nc.tensor.matmul(
                            sps[:, j * 512:(j + 1) * 512],
                            lhsT=qk8[4 + hp][p:p + 64, :, kt * 128:(kt + 1) * 128],
                            rhs=qk8[hp][p:p + 64, :, qc * 512:(qc + 1) * 512],
                            start=True, stop=True,
                            perf_mode=mybir.MatmulPerfMode.DoubleRow)
                    nc.scalar.activation(
                        exp_t[:, k0:k0 + n, :].rearrange("p a b -> p (a b)"),
                        sps[:, 0:n * 512], AF.Exp, scale=SCALE)
                    if prev is not None:
                        pqc = ((it - 1) % 8) // 2
                        if gi == 0:
                            emit_pv_qt(prev[0], prev[1], prev[2], 0,
                                       range(0, 8), pvT)
                        elif gi >= 2:
                            qt = gi - 2
                            if qt == 0:
                                emit_pv_qt(prev[0], prev[1], prev[2], 0,
                                           range(8, 16), pvT)
                            else:
                                emit_pv_qt(prev[0], prev[1], prev[2], qt,
                                           range(0, 16), pvT)
                            normalize_qt(prev[0], prev[1], pqc, qt, pvT)
                    pop(3 if it >= 26 else (2 if it < 6 else 1), it)
                    drain(it, gi)
                prev = (h2, hp, exp_t)

            # ---- epilogue ----
            wide[2] = True
            for qt in range(4):
                bank = (pvT, pjT)[qt % 2]
                emit_pv_qt(prev[0], prev[1], prev[2], qt, range(0, 16), bank)
                normalize_qt(prev[0], prev[1], 3, qt, bank)
            while pending:
                pending.pop(0)[1]()
    nc.compile()
    return nc


def _rope_tables_np():
    inv_freq = 1.0 / (10000.0 ** (np.arange(0, HD, 2, dtype=np.float32) / HD))
    t = np.arange(L, dtype=np.float32)
    freqs = np.outer(t, inv_freq).astype(np.float32)       # [L, 32]
    cos_h = np.cos(freqs).T                                # [32, L]
    sin_h = np.sin(freqs).T
    cosT = np.concatenate([cos_h, cos_h], 0)               # [64, L]
    sinT = np.concatenate([-sin_h, sin_h], 0)              # sign baked for rot trick
    return np.tile(cosT, (2, 1)), np.tile(sinT, (2, 1))    # [128, L]


_NC_CACHE = {}


def kernel(x, w_qkv, b_qkv, w_out, b_out):
    import ml_dtypes
    bf16 = ml_dtypes.bfloat16
    if "nc" not in _NC_CACHE:
        _NC_CACHE["nc"] = build_nc()
    nc = _NC_CACHE["nc"]

    cosT, sinT = _rope_tables_np()
    cosT = cosT.astype(bf16)
    sinT = sinT.astype(bf16)
    in_maps = []
    for c in range(8):
        b, g = divmod(c, 2)
        s = slice(512 * g, 512 * (g + 1))
        wqk = np.concatenate([w_qkv[0:D][s], w_qkv[D:2 * D][s]], 0)  # [1024, 1024]
        in_maps.append({
            "z8": np.zeros((128, L), dtype=ml_dtypes.float8_e4m3),
            "xT": np.ascontiguousarray(x[b].T).astype(bf16),
            "wqkT": np.ascontiguousarray(
                wqk.T.reshape(8, 128, 8, 128).transpose(2, 1, 0, 3)).astype(bf16),
            "wvT": np.ascontiguousarray(w_qkv[2 * D:3 * D][s].T).astype(bf16),
            "bqk": np.ascontiguousarray(
                np.concatenate([b_qkv[0:D][s], b_qkv[D:2 * D][s]])
                .reshape(8, 128).T).astype(np.float32),
            "bv": b_qkv[2 * D:3 * D][s][None].astype(bf16),
            "woT": np.ascontiguousarray(w_out[:, s].T).astype(bf16),
            "bout": (b_out if g == 0 else np.zeros_like(b_out))[None].astype(bf16),
            "cosT": cosT,
            "sinT": sinT,
        })
    res = run_bass_kernel_spmd(nc, in_maps, list(range(8)))
    _NC_CACHE["last_results"] = res
    parts = [np.asarray(r["out"], dtype=np.float32) for r in res.results]
    return np.stack([parts[2 * b] + parts[2 * b + 1] for b in range(4)])
